# revision 1
# baseline (speedup 1.0000x reference)
"""Block-sparse causal attention kernel for Trainium2 (8 NeuronCores).

Problem: B=2, T=2048, H=16, Dqk=Dv=128, fp16, BLOCK 64x64 block mask +
causal, softmax over keys.

Sharding: the 32 (b, h) pairs are split 4-per-core across 8 cores (data +
head parallel); no cross-core communication.

Per-core device algorithm (per (b,h) pair):
  - Load Q^T, K^T as [d=128, t=2048] via HW DMA transpose; V natural
    [t mod 128 -> partition, 16 k-tiles, d].
  - Loop q-groups g (512 queries each), inner k-tiles kt (128 keys):
      S^T[n,m] = K_kt @ Q_g^T  on PE (out [128 keys, 512 queries] psum)
      P^T = exp(S^T * 1/sqrt(d))  on ACT (fp16 to SBUF), no max-subtraction
      P^T *= blockmask (broadcast 0/1, DVE); diagonal chunk *= causal 0/1
      O^T[d,m] += V_kt^T @ P^T  on PE (accumulate over kt in psum)
      l[1,m]  += ones^T @ P^T   on PE (softmax denominator)
  - Readout: evacuate O^T (unnormalized) and l to DRAM; the host fuses the
    final [d, t] -> [t, d] transpose with the 1/l softmax normalization.

The block mask is applied multiplicatively after exp (scores are O(5), so
exp never overflows), which keeps the program identical across all cores:
the mask enters only as data (a per-(bh, step) 0/1 table plus one shared
in-block causal-triangle tile), so SPMD holds even though each core sees
different masks. Fully-masked below-diagonal chunks are skipped via
suffix-trimmed matmul/exp/mask widths.
"""

import numpy as np

import concourse.bass as bass
import concourse.mybir as mybir
import concourse.tile as tile
from concourse import bacc

B, T, H, D = 2, 2048, 16, 128
BM = 64           # mask block size
NT = T // 128     # 16 k-tiles / q-tiles of 128
NG = 4            # q-groups of 512 queries
BH_PER_CORE = 4
N_CORES = 8
SCALE = float(1.0 / np.sqrt(D))

F16 = mybir.dt.float16
F32 = mybir.dt.float32

# step s enumerates (g, kt): for g in 0..3: for kt in 0..4g+3
STEP_OFF = [0, 4, 12, 24]
N_STEPS = 40


def build_program(loop_n=None):
    nc = bacc.Bacc("TRN2", target_bir_lowering=False, debug=False)

    q_d = nc.dram_tensor("q", (BH_PER_CORE, T, D), F16, kind="ExternalInput")
    k_d = nc.dram_tensor("k", (BH_PER_CORE, T, D), F16, kind="ExternalInput")
    v_d = nc.dram_tensor("v", (BH_PER_CORE, T, D), F16, kind="ExternalInput")
    # mask table: one 0/1 value per (step, k-half-row, q-block), stored
    # duplicated in adjacent pairs so the broadcast AP keeps a packed last
    # dim (enables the DVE 2x perf mode)
    m8_d = nc.dram_tensor(
        "m8", (BH_PER_CORE, 128, N_STEPS * 16), F16, kind="ExternalInput"
    )
    c01_d = nc.dram_tensor("c01", (128, 128), F16, kind="ExternalInput")
    # o is stored transposed ([d, t] per pair); host does the final
    # [d, t] -> [t, d] transpose fused with the 1/l normalization
    o_d = nc.dram_tensor("o", (BH_PER_CORE, D, T), F16, kind="ExternalOutput")
    l_d = nc.dram_tensor("l", (BH_PER_CORE, T), F32, kind="ExternalOutput")

    with tile.TileContext(nc) as tc:
        with (
            tc.tile_pool(name="inp", bufs=4) as inp,
            tc.tile_pool(name="const", bufs=1) as cpool,
            tc.tile_pool(name="pt", bufs=6) as ppool,
            tc.tile_pool(name="outp", bufs=6) as opool,
            tc.tile_pool(name="sc", bufs=3, space="PSUM") as scpool,
            tc.tile_pool(name="ot", bufs=1, space="PSUM") as otpool,
            tc.tile_pool(name="lp", bufs=1, space="PSUM") as lpool,
        ):
            ones = cpool.tile([128, 1], F16)
            nc.vector.memset(ones[:, :], 1.0)
            c01 = cpool.tile([128, 128], F16)
            nc.sync.dma_start(c01[:, :], c01_d.ap()[:, :])

            if loop_n is not None:
                loop_cm = tc.For_i(
                    0,
                    loop_n,
                    1,
                    hint_engines=(
                        mybir.EngineType.PE,
                        mybir.EngineType.Activation,
                        mybir.EngineType.DVE,
                        mybir.EngineType.SP,
                        mybir.EngineType.Pool,
                    ),
                )
                loop_cm.__enter__()

            tiles = []
            for bh in range(BH_PER_CORE):
                qT = inp.tile([128, T], F16, tag="qT")
                nc.sync.dma_start_transpose(qT[:, :], q_d.ap()[bh])
                kT = inp.tile([128, T], F16, tag="kT")
                nc.sync.dma_start_transpose(kT[:, :], k_d.ap()[bh])
                v = inp.tile([128, NT, 128], F16, tag="v")
                nc.sync.dma_start(
                    v[:, :, :], v_d.ap()[bh].rearrange("(nt p) d -> p nt d", p=128)
                )
                m8 = inp.tile([128, N_STEPS * 16], F16, tag="m8")
                nc.sync.dma_start(m8[:, :], m8_d.ap()[bh])
                tiles.append((qT, kT, v, m8))

            for bh in range(BH_PER_CORE):
                qT, kT, v, m8 = tiles[bh]
                for g in range(NG):
                    nkt = 4 * g + 4
                    ot = otpool.tile([128, 512], F32)   # O^T accum [d, m]
                    lps = lpool.tile([1, 512], F32)     # l accum [1, m]

                    for kt0 in range(0, nkt, 2):
                        # causal suffix-trim: columns below the diagonal chunk
                        # are fully masked; skip them
                        offs = [
                            max(0, kt0 + h - 4 * g) * 128 if kt0 + h > 4 * g else 0
                            for h in range(2)
                        ]
                        sc = scpool.tile([128, 1024], F32)  # 2 psum banks
                        for h in range(2):
                            kt = kt0 + h
                            o0 = offs[h]
                            nc.tensor.matmul(
                                sc[:, h * 512 + o0 : (h + 1) * 512],
                                lhsT=kT[:, kt * 128 : (kt + 1) * 128],
                                rhs=qT[:, g * 512 + o0 : (g + 1) * 512],
                                start=True,
                                stop=True,
                            )
                        pt = ppool.tile([128, 1024], F16)
                        s0 = STEP_OFF[g] + kt0
                        if offs[0] == 0 and offs[1] == 0:
                            nc.scalar.activation(
                                pt[:, :],
                                sc[:, :],
                                mybir.ActivationFunctionType.Exp,
                                scale=SCALE,
                            )
                            nc.vector.tensor_mul(
                                pt[:, :],
                                pt[:, :],
                                m8[:, s0 * 16 : (s0 + 2) * 16]
                                .rearrange("p (j t) -> p j t", t=2)
                                .broadcast_to([128, 16, 2, 32])
                                .rearrange("p j t r -> p j r t"),
                            )
                        else:
                            for h in range(2):
                                o0 = h * 512 + offs[h]
                                w = 512 - offs[h]
                                nb = w // 64
                                nc.scalar.activation(
                                    pt[:, o0 : o0 + w],
                                    sc[:, o0 : o0 + w],
                                    mybir.ActivationFunctionType.Exp,
                                    scale=SCALE,
                                )
                                nc.vector.tensor_mul(
                                    pt[:, o0 : o0 + w],
                                    pt[:, o0 : o0 + w],
                                    m8[
                                        :,
                                        (s0 + h) * 16 + 2 * (offs[h] // 64)
                                        : (s0 + h + 1) * 16,
                                    ]
                                    .rearrange("p (j t) -> p j t", t=2)
                                    .broadcast_to([128, nb, 2, 32])
                                    .rearrange("p j t r -> p j r t"),
                                )
                        # in-chunk causal triangle on diagonal chunks (on the
                        # otherwise-idle gpsimd engine)
                        for h in range(2):
                            kt = kt0 + h
                            if 4 * g <= kt <= 4 * g + 3:
                                c0 = h * 512 + (kt - 4 * g) * 128
                                nc.gpsimd.tensor_mul(
                                    pt[:, c0 : c0 + 128],
                                    pt[:, c0 : c0 + 128],
                                    c01[:, :],
                                )
                        for h in range(2):
                            kt = kt0 + h
                            o0 = offs[h]
                            nc.tensor.matmul(
                                ot[:, o0:],
                                lhsT=v[:, kt, :],
                                rhs=pt[:, h * 512 + o0 : (h + 1) * 512],
                                start=(kt == 0),
                                stop=(kt == nkt - 1),
                            )
                            nc.tensor.matmul(
                                lps[:, o0:],
                                lhsT=ones[:, :],
                                rhs=pt[:, h * 512 + o0 : (h + 1) * 512],
                                start=(kt == 0),
                                stop=(kt == nkt - 1),
                            )

                    lsb = opool.tile([1, 512], F32, tag="lsb")
                    nc.vector.tensor_copy(lsb[:, :], lps[:, :])
                    nc.sync.dma_start(
                        l_d.ap()[bh : bh + 1, g * 512 : (g + 1) * 512], lsb[0:1, :]
                    )
                    otn = opool.tile([128, 512], F16, tag="otn")
                    nc.vector.tensor_copy(otn[:, :], ot[:, :])
                    nc.sync.dma_start(
                        o_d.ap()[bh][:, g * 512 : (g + 1) * 512], otn[:, :]
                    )

            if loop_n is not None:
                loop_cm.__exit__(None, None, None)

    nc.compile()
    return nc


def make_host_inputs(q, k, v, block_mask):
    """Split full inputs into 8 per-core input maps (4 (b,h) pairs each)."""
    q, k, v = np.asarray(q), np.asarray(k), np.asarray(v)
    block_mask = np.asarray(block_mask)
    pairs = [(b, h) for b in range(B) for h in range(H)]
    kb_idx = np.arange(32)
    vis_causal = kb_idx[:, None] <= kb_idx[None, :]  # [kb, qb]
    c01 = (np.arange(128)[None, :] >= np.arange(128)[:, None]).astype(np.float16)

    in_maps = []
    for c in range(N_CORES):
        sel = pairs[c * BH_PER_CORE : (c + 1) * BH_PER_CORE]
        qc = np.stack([q[b, :, h, :] for b, h in sel])
        kc = np.stack([k[b, :, h, :] for b, h in sel])
        vc = np.stack([v[b, :, h, :] for b, h in sel])
        m8 = np.zeros((BH_PER_CORE, 128, N_STEPS * 16), np.float16)
        for i, (b, h) in enumerate(sel):
            # block_mask[b, h] is [q_block, k_block]; vis is [kb, qb]
            vis = (block_mask[b, h].T & vis_causal).astype(np.float16)
            for g in range(NG):
                for kt in range(4 * g + 4):
                    s = STEP_OFF[g] + kt
                    for half in range(2):
                        kb = 2 * kt + half
                        m8[
                            i, half * 64 : (half + 1) * 64, s * 16 : (s + 1) * 16
                        ] = np.repeat(vis[kb, 8 * g : 8 * g + 8], 2)[None, :]
        in_maps.append(
            {
                "q": np.ascontiguousarray(qc),
                "k": np.ascontiguousarray(kc),
                "v": np.ascontiguousarray(vc),
                "m8": m8,
                "c01": c01,
            }
        )
    return in_maps


_NC_CACHE = {}


def get_program():
    if "nc" not in _NC_CACHE:
        _NC_CACHE["nc"] = build_program()
    return _NC_CACHE["nc"]


def kernel(q, k, v, block_mask, _trace=False):
    from concourse.bass_utils import run_bass_kernel_spmd

    nc = get_program()
    in_maps = make_host_inputs(q, k, v, block_mask)
    res = run_bass_kernel_spmd(
        nc, in_maps, core_ids=list(range(N_CORES)), trace=_trace
    )
    pairs = [(b, h) for b in range(B) for h in range(H)]
    out = np.zeros((B, T, H, D), np.float16)
    for c in range(N_CORES):
        sel = pairs[c * BH_PER_CORE : (c + 1) * BH_PER_CORE]
        oc = res.results[c]["o"]  # [bh, d, t] transposed-unnormalized
        lc = res.results[c]["l"]
        for i, (b, h) in enumerate(sel):
            out[b, :, h, :] = (
                oc[i].T.astype(np.float32) / lc[i][:, None]
            ).astype(np.float16)
    if _trace:
        return out, res
    return out

